# revision 37
# baseline (speedup 1.0000x reference)
"""BatchedSharedLoRA TRN2 kernel (final: ~189 us vs 262 us baseline).

Math (per adapter a):  out[a] = x + SCALING * u / (||u||_rows + EPS),
where u = (x @ A_a) @ B_a,  x:[M,H], A:[H,R], B:[R,H].

Sharding: DATA-parallel over rows -- core i owns rows [i*512, (i+1)*512) of
the flattened x [4096, 4096] and computes all 8 adapters for its slice.

Design (what mattered, in order):
  * fp16 OUTPUT (halves the dominant 64 MiB/core output write; host upcasts
    to f32; rel-err gate 2e-2 >> fp16 rounding ~1e-3).
  * x is transposed + cast on the host: xT bf16 feeds mm1, x fp16 feeds the
    residual adds. Removes all on-chip PE transposes of x.
  * Adapter-PAIR packing: mm1 computes tT for two adapters per pass
    (A-pair weight [128h, 128r2] fills the PE array); mm2 places the pair
    on PE row-groups 0-1 / 2-3 (row_grps emitted, though streams serialize).
  * mm1 for pair p+2 runs as a DENSE 32-matmul block inside pair p's body
    (two-pair pipeline distance) so its tT2-evac / norm chain never stalls
    the PE queue on ACT/DVE backlog, and the dense block also pipelines the
    following mm2 matmuls.
  * Row norms via one fused matmul per m-block: rhs = [BBT2 | I] yields
    g = t@BBT (block-diag, both adapters) AND t = transpose(tT) in a single
    N=256 matmul; DVE affine_mul_reduce then gives ||u||^2 per row.
  * Residual out = s*u + x split to balance engines (~24 'A' / 8 'B' units):
      'A': ACT activation evac with per-partition scale (v = s*u, fp16) +
           DVE tensor_add (fp16 2x_1p mode) -- NOT scalar_tensor_tensor /
           affine_then_add, which are locked to 1x.
      'B': DVE scalar_tensor_tensor fused (u*s + x) straight from PSUM (1x).
    GPSIMD adds were tried and REMOVED: a GpSimd TENSOR_TENSOR concurrent
    with a DVE TENSOR_TENSOR on the same x tile serializes the DVE op 4x
    (SBUF interference) and stalls the PE cold.
  * PSUM: u-ring bufs=3 ([128,1024] f32) is the PE<->consumer slack that
    keeps mm2 matmuls pipelined; tT2_ps bufs=1; gt in one bank (2-step).
  * The PE gets activity-throttled to K=4/8 (1.2 GHz) after ~75 us of load
    and never recovers (HW governor; keep-warm filler matmuls and dense
    re-warm blocks both failed to prevent it) -- so the design minimizes
    PE instructions rather than chasing HAM warmth. fp8 DoubleRow for mm1
    was tried and was SLOWER at cold clocks (LDW overhead).

Per-core HBM traffic: 4 (x fp16) + 4 (xT bf16) + 4 (A) + 4 (B) + 32 (out
fp16) ~= 48 MiB -> ~140 us roofline at 358 GB/s.
"""

import numpy as np
import ml_dtypes

import concourse.bass as bass
import concourse.mybir as mybir
import concourse.tile as tile
from concourse import bacc, bass_utils

NADAPT = 8
BATCH, SEQ, H, R = 2, 2048, 4096, 64
M = BATCH * SEQ  # 4096
SCALING = 2.0
EPS = 1e-8

F32 = mybir.dt.float32
BF16 = mybir.dt.bfloat16
FP16 = mybir.dt.float16

MROWS = M // 8  # 512 rows per core
NBLK = MROWS // 128  # 4 m-blocks per core
KH = H // 128  # 32 contraction chunks for mm1
NPAIR = NADAPT // 2  # 4 adapter pairs

# Per-pair residual-unit engine pattern, indexed by (j*2 + e).
#   A: ACT evac + DVE tensor_add;  B: DVE fused from PSUM.
# GPSIMD adds were tried and removed: a GpSimd TENSOR_TENSOR running
# concurrently with a DVE TENSOR_TENSOR on the same x tile serializes the
# DVE op 4x (SBUF interference) and stalls the PE into HAM-cold.
# Each j gets at most one B so the two chunk-evacs of an iter can run on
# ACT and DVE in parallel.
UNIT_PATTERN = {
    0: "ABAAABAB",  # j0 (A,B), j1 (A,A), j2 (A,B), j3 (A,B)
    1: "ABAAABAB",
}


def build_kernel() -> bass.Bass:
    nc = bacc.Bacc(trn_type="TRN2")
    xr_d = nc.dram_tensor("xr", [MROWS, H], FP16, kind="ExternalInput")
    xt_d = nc.dram_tensor("xt", [128, KH * MROWS], BF16, kind="ExternalInput")
    a2_d = nc.dram_tensor("a2", [NPAIR * 128, KH * 128], BF16, kind="ExternalInput")
    b2_d = nc.dram_tensor("b2", [NPAIR * 128, H], BF16, kind="ExternalInput")
    bbtI_d = nc.dram_tensor("bbtI", [NPAIR * 128, 256], BF16, kind="ExternalInput")
    out_d = nc.dram_tensor("out", [NADAPT * MROWS, H], FP16, kind="ExternalOutput")

    with tile.TileContext(nc) as tc:
        with (
            tc.tile_pool(name="xpool", bufs=NBLK) as xpool,
            tc.tile_pool(name="xtpool", bufs=NBLK) as xtpool,
            tc.tile_pool(name="a2_pool", bufs=4) as a2_pool,
            tc.tile_pool(name="b2_pool", bufs=2) as b2_pool,
            tc.tile_pool(name="bbtI_pool", bufs=4) as bbtI_pool,
            tc.tile_pool(name="tT2_sb_pool", bufs=4) as tT2_sb_pool,
            tc.tile_pool(name="t2_sb_pool", bufs=2) as t2_sb_pool,
            tc.tile_pool(name="junk_pool", bufs=2) as junk_pool,
            tc.tile_pool(name="stat_pool", bufs=4) as stat_pool,
            tc.tile_pool(name="v_pool", bufs=3) as v_pool,
            tc.tile_pool(name="out_pool", bufs=4) as out_pool,
            tc.tile_pool(name="tT2_ps_pool", bufs=1, space="PSUM") as tT2_ps_pool,
            tc.tile_pool(name="u_ps_pool", bufs=3, space="PSUM") as u_ps_pool,
            tc.tile_pool(name="gt_ps_pool", bufs=1, space="PSUM") as gt_ps_pool,
        ):
            x_tiles = [
                xpool.tile([128, H], FP16, name=f"x_sb_{j}", tag="x_sb")
                for j in range(NBLK)
            ]

            xt_tiles = [
                xtpool.tile([128, KH // NBLK, MROWS], BF16, name=f"xt_{g}", tag="xt")
                for g in range(NBLK)
            ]

            def load_a2(p):
                a2_sb = a2_pool.tile([128, KH, 128], BF16, name=f"a2_{p}", tag="a2")
                nc.sync.dma_start(
                    out=a2_sb,
                    in_=a2_d.ap()[p * 128 : (p + 1) * 128, :].rearrange(
                        "p (k r) -> p k r", r=128
                    ),
                )
                return a2_sb

            def load_b2(p):
                b2_sb = b2_pool.tile([128, H], BF16, name=f"b2_{p}", tag="b2")
                nc.sync.dma_start(out=b2_sb, in_=b2_d.ap()[p * 128 : (p + 1) * 128, :])
                return b2_sb

            def load_bbtI(p):
                bbtI_sb = bbtI_pool.tile([128, 256], BF16, name=f"bbtI_{p}", tag="bbtI")
                nc.sync.dma_start(
                    out=bbtI_sb, in_=bbtI_d.ap()[p * 128 : (p + 1) * 128, :]
                )
                return bbtI_sb

            def mm1_block(p, a2_sb):
                """Dense 32-matmul mm1 for pair p: tT2 = A2_p^T @ x^T."""
                tT2_ps = tT2_ps_pool.tile(
                    [128, MROWS], F32, name=f"tT2_ps_{p}", tag="tT2_ps"
                )
                for k in range(KH):
                    nc.tensor.matmul(
                        tT2_ps,
                        a2_sb[:, k, :],
                        xt_tiles[k // 8][:, k % 8, :],
                        start=(k == 0),
                        stop=(k == KH - 1),
                    )
                return tT2_ps

            def norm_part(p, jh, tT2_ps, bbtI_sb, st):
                """Half of the row-norm chain for pair p (j-blocks 2jh, 2jh+1).

                Split in two so the 1-bank gt ring never blocks the PE on a
                DVE backlog, and so the halves can be emitted a few mm2
                iterations apart. jh==0 also evacuates tT2; jh==1 finishes
                s = 2/(||u||+EPS)."""
                if jh == 0:
                    st["tT2"] = tT2_sb_pool.tile(
                        [128, MROWS], BF16, name=f"tT2_{p}", tag="tT2"
                    )
                    nc.scalar.copy(out=st["tT2"], in_=tT2_ps)
                    st["t2"] = t2_sb_pool.tile(
                        [128, NBLK, 128], BF16, name=f"t2_{p}", tag="t2"
                    )
                    st["ssq8"] = stat_pool.tile(
                        [128, 2 * NBLK], F32, name=f"ssq8_{p}", tag="ssq8"
                    )
                tT2_bf, t2_all, ssq8 = st["tT2"], st["t2"], st["ssq8"]
                gt_ps = gt_ps_pool.tile(
                    [128, 2, 256], F32, name=f"gt_ps_{p}_{jh}", tag="gt"
                )
                for jj in range(2):
                    j = jh * 2 + jj
                    nc.tensor.matmul(
                        gt_ps[:, jj, :],
                        tT2_bf[:, j * 128 : (j + 1) * 128],
                        bbtI_sb,
                        start=True,
                        stop=True,
                    )
                nc.scalar.copy(
                    out=t2_all[:, jh * 2 : jh * 2 + 2, :],
                    in_=gt_ps[:, :, 128:256],
                )
                for jj in range(2):
                    for e in range(2):
                        j = jh * 2 + jj
                        junk = junk_pool.tile(
                            [128, R], BF16, name=f"junk_{p}_{j}_{e}", tag="junk"
                        )
                        c = j * 2 + e
                        nc.vector.affine_mul_reduce(
                            out=junk,
                            accum_out=ssq8[:, c : c + 1],
                            in0=gt_ps[:, jj, e * R : (e + 1) * R],
                            in1=t2_all[:, j, e * R : (e + 1) * R],
                            scale=1.0,
                            bias=0.0,
                        )
                if jh == 1:
                    # nh = 0.5*||u|| + 0.5*EPS;  s = 1/nh = 2/(||u||+EPS)
                    nh8 = stat_pool.tile(
                        [128, 2 * NBLK], F32, name=f"nh8_{p}", tag="nh8"
                    )
                    nc.scalar.activation(
                        out=nh8, in_=ssq8, func=mybir.ActivationFunctionType.Sqrt,
                        scale=0.25,
                    )
                    nc.vector.tensor_scalar_add(out=nh8, in0=nh8, scalar1=EPS * 0.5)
                    s8 = stat_pool.tile(
                        [128, 2 * NBLK], F32, name=f"s8_{p}", tag="s8"
                    )
                    nc.vector.reciprocal(out=s8, in_=nh8)
                    st["s8"] = s8

            def mm2_jblock(p, j, st, b2_sb, dma_ctr):
                """mm2 + residual + out-DMA for pair p, m-block j."""
                pat = UNIT_PATTERN[p % 2]
                tT2_bf, s8 = st["tT2"], st["s8"]
                out_sbs = [None, None]
                v4s = [None, None]
                for e in range(2):
                    a = 2 * p + e
                    out_sbs[e] = out_pool.tile(
                        [128, H], FP16, name=f"out_{a}_{j}", tag="out"
                    )
                    if pat[j * 2 + e] == "A":
                        v4s[e] = v_pool.tile(
                            [128, H], FP16, name=f"v_{a}_{j}", tag="v"
                        )
                for n in range(4):
                    u_ps = [None, None]
                    for e in range(2):
                        u_ps[e] = u_ps_pool.tile(
                            [128, 1024], F32, name=f"u_{p}_{j}_{n}_{e}", tag="u"
                        )
                    # pair matmuls: adapter a on PE rows 0-63, adapter b on
                    # rows 64-127. Same-weight MMs adjacent to cut LDW churn.
                    # (N=1024 single matmuls fail neuronxcc: the N<=512 fp32
                    # PSUM-bank limit is enforced.)
                    for e in range(2):
                        for half in range(2):
                            c0 = n * 1024 + half * 512
                            nc.tensor.matmul(
                                u_ps[e][:, half * 512 : (half + 1) * 512],
                                tT2_bf[e * 64 : (e + 1) * 64, j * 128 : (j + 1) * 128],
                                b2_sb[e * 64 : (e + 1) * 64, c0 : c0 + 512],
                                start=True,
                                stop=True,
                            )
                    # residual: out = s*u + x
                    for e in range(2):
                        c = j * 2 + e
                        kind = pat[c]
                        xj = x_tiles[j][:, n * 1024 : (n + 1) * 1024]
                        if kind == "A":
                            nc.scalar.mul(
                                out=v4s[e][:, n * 1024 : (n + 1) * 1024],
                                in_=u_ps[e],
                                mul=s8[:, c : c + 1],
                            )
                        else:
                            nc.vector.scalar_tensor_tensor(
                                out=out_sbs[e][:, n * 1024 : (n + 1) * 1024],
                                in0=u_ps[e],
                                scalar=s8[:, c : c + 1],
                                in1=xj,
                                op0=mybir.AluOpType.mult,
                                op1=mybir.AluOpType.add,
                            )
                    if n == 3:
                        for e in range(2):
                            kind = pat[j * 2 + e]
                            if kind == "A":
                                nc.vector.tensor_add(out_sbs[e], v4s[e], x_tiles[j])
                            r0 = (2 * p + e) * MROWS + j * 128
                            dma_ctr[0] += 1
                            nc.gpsimd.dma_start(
                                out=out_d.ap()[r0 : r0 + 128, :], in_=out_sbs[e]
                            )

            # ---- Input DMAs, ordered so each mm1 block's a2 and the x tile
            # for each residual j-block land just in time.
            a2_sbs = {0: load_a2(0)}
            for g in range(NBLK):
                nc.sync.dma_start(
                    out=xt_tiles[g],
                    in_=xt_d.ap()[
                        :, g * (KH // NBLK) * MROWS : (g + 1) * (KH // NBLK) * MROWS
                    ].rearrange("p (k m) -> p k m", m=MROWS),
                )
            a2_sbs[1] = load_a2(1)
            b2_sbs = {0: load_b2(0)}
            bbtI_sbs = {0: load_bbtI(0), 1: load_bbtI(1)}
            nc.sync.dma_start(out=x_tiles[0], in_=xr_d.ap()[0:128, :])
            a2_sbs[2] = load_a2(2)
            a2_sbs[3] = load_a2(3)
            for j in range(1, NBLK):
                nc.sync.dma_start(
                    out=x_tiles[j], in_=xr_d.ap()[j * 128 : (j + 1) * 128, :]
                )
            bbtI_sbs[2] = load_bbtI(2)
            bbtI_sbs[3] = load_bbtI(3)

            # ---- Fused schedule: ALL FOUR mm1 blocks run before the PE's
            # ~75us activity clamp kicks in (2x clock), woven between pair
            # 0's mm2 j-blocks so the ACT/DVE consumers start early and the
            # norm-chain halves never head-of-line-block the PE queue.
            sts = {q: {} for q in range(NPAIR)}
            tT2_pss = {}
            dma_ctr = [0]

            tT2_pss[0] = mm1_block(0, a2_sbs[0])
            norm_part(0, 0, tT2_pss[0], bbtI_sbs[0], sts[0])
            norm_part(0, 1, tT2_pss[0], bbtI_sbs[0], sts[0])
            tT2_pss[1] = mm1_block(1, a2_sbs[1])
            norm_part(1, 0, tT2_pss[1], bbtI_sbs[1], sts[1])
            norm_part(1, 1, tT2_pss[1], bbtI_sbs[1], sts[1])
            b2_sbs[1] = load_b2(1)

            mm2_jblock(0, 0, sts[0], b2_sbs[0], dma_ctr)
            tT2_pss[2] = mm1_block(2, a2_sbs[2])
            norm_part(2, 0, tT2_pss[2], bbtI_sbs[2], sts[2])
            mm2_jblock(0, 1, sts[0], b2_sbs[0], dma_ctr)
            norm_part(2, 1, tT2_pss[2], bbtI_sbs[2], sts[2])
            mm2_jblock(0, 2, sts[0], b2_sbs[0], dma_ctr)
            tT2_pss[3] = mm1_block(3, a2_sbs[3])
            norm_part(3, 0, tT2_pss[3], bbtI_sbs[3], sts[3])
            mm2_jblock(0, 3, sts[0], b2_sbs[0], dma_ctr)
            norm_part(3, 1, tT2_pss[3], bbtI_sbs[3], sts[3])

            for p in range(1, NPAIR):
                if p + 1 < NPAIR:
                    b2_sbs[p + 1] = load_b2(p + 1)
                for j in range(NBLK):
                    mm2_jblock(p, j, sts[p], b2_sbs[p], dma_ctr)

    nc.compile()
    return nc


_NC_CACHE = {}


def _get_nc():
    if "nc" not in _NC_CACHE:
        _NC_CACHE["nc"] = build_kernel()
    return _NC_CACHE["nc"]


def _prep_inputs(x, lora_A, lora_B):
    xm = np.ascontiguousarray(np.asarray(x, dtype=np.float32)).reshape(M, H)
    lora_A = np.asarray(lora_A, dtype=np.float32)
    lora_B = np.asarray(lora_B, dtype=np.float32)
    assert lora_A.shape == (NADAPT, H, R) and lora_B.shape == (NADAPT, R, H)
    bf = ml_dtypes.bfloat16

    # A pairs: a2[pair*128 + p, k*128 + e*64 + r] = A[2*pair+e, k*128+p, r]
    a2 = np.ascontiguousarray(
        lora_A.astype(bf).reshape(NPAIR, 2, KH, 128, R).transpose(0, 3, 2, 1, 4)
    ).reshape(NPAIR * 128, KH * 128)
    # B pairs: b2[pair*128 + e*64 + r, h] = B[2*pair+e, r, h]
    b2 = np.ascontiguousarray(lora_B.astype(bf).reshape(NPAIR * 128, H))
    # BBT from the bf16-rounded B (consistent with mm2), block-diag per pair,
    # with an identity appended so one matmul yields both g = t@BBT and t.
    Bf = b2.astype(np.float32).reshape(NADAPT, R, H)
    bbt = np.einsum("arh,ash->ars", Bf, Bf)
    bbtI = np.zeros((NPAIR, 128, 256), np.float32)
    bbtI[:, 0:R, 0:R] = bbt[0::2]
    bbtI[:, R:128, R:128] = bbt[1::2]
    bbtI[:, :, 128:256] = np.eye(128, dtype=np.float32)[None]
    bbtI = np.ascontiguousarray(bbtI.astype(bf).reshape(NPAIR * 128, 256))

    x16 = xm.astype(np.float16)
    xtg = np.ascontiguousarray(xm.T).astype(bf)  # [H, M]
    return x16, xtg, a2, b2, bbtI


def run(inputs: dict, trace: bool = False):
    """Returns (output [8, 2, 2048, 4096] f32, BassKernelResults)."""
    x16, xtg, a2, b2, bbtI = _prep_inputs(
        inputs["x"], inputs["lora_A"], inputs["lora_B"]
    )

    nc = _get_nc()
    in_maps = []
    xtg_k = xtg.reshape(KH, 128, M)
    for i in range(8):
        xt_c = np.ascontiguousarray(
            xtg_k[:, :, i * MROWS : (i + 1) * MROWS].transpose(1, 0, 2)
        ).reshape(128, KH * MROWS)
        in_maps.append(
            {
                "xr": x16[i * MROWS : (i + 1) * MROWS],
                "xt": xt_c,
                "a2": a2,
                "b2": b2,
                "bbtI": bbtI,
            }
        )
    res = bass_utils.run_bass_kernel_spmd(
        nc, in_maps, core_ids=list(range(8)), trace=trace
    )
    # core i returns [NADAPT*MROWS, H] fp16 for its row slice; reassemble.
    parts = [r["out"].reshape(NADAPT, MROWS, H) for r in res.results]
    out = (
        np.concatenate(parts, axis=1).astype(np.float32).reshape(NADAPT, BATCH, SEQ, H)
    )
    return out, res


def kernel(x, lora_A, lora_B):
    out, _ = run({"x": x, "lora_A": lora_A, "lora_B": lora_B})
    return out
